# revision 16
# baseline (speedup 1.0000x reference)
"""PRXAttention TRN2 kernel: 8-core SPMD (2 batches x 4 head-groups).

Per core (b, g): project q/k/v for 4 heads (img) + k/v (txt), RMSNorm via
PE ones-matmul partition reduction, RoPE with host-prepared pair-deinterleaved
tables (g_q/g_k folded in), softmax without max-subtraction (scores bounded),
masking via host-side zeroing of masked encoder tokens + Z correction,
partial out-projection for the 4 heads; host sums the 4 partials per batch.
"""

import numpy as np
import ml_dtypes

bf16 = ml_dtypes.bfloat16

B, L_IMG, L_TXT = 2, 2048, 2048 // 4
D, H, DH = 2048, 16, 128
HPC = 4                      # heads per core
NCORES = 8
EPS = 1e-6
SM_SCALE = 1.0 / float(np.sqrt(DH))
NDT = D // 128               # 16 d-model tiles
NLC = L_IMG // 512           # 4 img l-chunks of 512
NKT_TXT = L_TXT // 128       # 4 txt key tiles
NKT = NKT_TXT + L_IMG // 128  # 20 key tiles of 128
PIPE = 3                     # attention S->exp software pipeline depth

_PROG = None


def _build_program():
    from contextlib import ExitStack

    import concourse.bacc as bacc
    import concourse.bass as bass
    import concourse.tile as tile
    from concourse import mybir

    f32 = mybir.dt.float32
    bf = mybir.dt.bfloat16
    AF = mybir.ActivationFunctionType

    nc = bacc.Bacc("TRN2", target_bir_lowering=False)
    xt_d = nc.declare_dram_parameter("xt", [D, L_IMG], bf, isOutput=False)
    et_d = nc.declare_dram_parameter("et", [D, L_TXT], bf, isOutput=False)
    wq_d = nc.declare_dram_parameter("wq", [D, HPC * DH], bf, isOutput=False)
    wk_d = nc.declare_dram_parameter("wk", [D, HPC * DH], bf, isOutput=False)
    wv_d = nc.declare_dram_parameter("wv", [D, HPC * DH], bf, isOutput=False)
    wtk_d = nc.declare_dram_parameter("wtk", [D, HPC * DH], bf, isOutput=False)
    wtv_d = nc.declare_dram_parameter("wtv", [D, HPC * DH], bf, isOutput=False)
    wo_d = nc.declare_dram_parameter("wo", [HPC * DH, D], bf, isOutput=False)
    tq_d = nc.declare_dram_parameter("tq", [DH, 2, L_IMG], bf, isOutput=False)
    tk_d = nc.declare_dram_parameter("tk", [DH, 2, L_IMG], bf, isOutput=False)
    gtk_d = nc.declare_dram_parameter("gtk", [DH, 1], f32, isOutput=False)
    nm_d = nc.declare_dram_parameter("nm", [1, 1], f32, isOutput=False)
    out_d = nc.declare_dram_parameter("out", [L_IMG, D], f32, isOutput=True)

    with tile.TileContext(nc) as tc, ExitStack() as ctx:
        # ---- persistent pools (whole kernel) ----
        const = ctx.enter_context(tc.tile_pool(name="const", bufs=1))
        persist = ctx.enter_context(tc.tile_pool(name="persist", bufs=1))
        ps_s = ctx.enter_context(
            tc.tile_pool(name="ps_s", bufs=PIPE + 1, space=bass.MemorySpace.PSUM))
        ps_acc = ctx.enter_context(
            tc.tile_pool(name="ps_acc", bufs=2, space=bass.MemorySpace.PSUM))
        ps_z = ctx.enter_context(
            tc.tile_pool(name="ps_z", bufs=1, space=bass.MemorySpace.PSUM))
        ps_b = ctx.enter_context(
            tc.tile_pool(name="ps_b", bufs=1, space=bass.MemorySpace.PSUM))

        ones_col = const.tile([128, 1], bf, name="ones_col")
        nc.vector.memset(ones_col[:], 1.0)
        ones_row = const.tile([1, 128], f32, name="ones_row")
        nc.vector.memset(ones_row[:], 1.0)
        gtk_s = const.tile([DH, 1], f32, name="gtk_s")
        nc.gpsimd.dma_start(gtk_s[:], gtk_d[:, :])
        nm_s = const.tile([1, 1], f32, name="nm_s")
        nc.gpsimd.dma_start(nm_s[:], nm_d[:, :])
        eps_s = const.tile([1, 1], f32, name="eps_s")
        nc.vector.memset(eps_s[:], EPS)
        tqs = const.tile([DH, 2, L_IMG], bf, name="tqs")
        nc.gpsimd.dma_start(tqs[:], tq_d[:, :, :])
        tks_tab = const.tile([DH, 2, L_IMG], bf, name="tks_tab")
        nc.gpsimd.dma_start(tks_tab[:], tk_d[:, :, :])

        qf = [persist.tile([DH, L_IMG], bf, name=f"qf{h}", tag=f"qf{h}")
              for h in range(HPC)]
        kf = [persist.tile([DH, L_IMG], bf, name=f"kf{h}", tag=f"kf{h}")
              for h in range(HPC)]
        tkf = [persist.tile([DH, L_TXT], bf, name=f"tkf{h}", tag=f"tkf{h}")
               for h in range(HPC)]
        vs = persist.tile([128, NKT, HPC * DH], bf, name="vs")

        def rmsnorm_factor(pool_small, acc_psum):
            """acc_psum: [128, n] f32 projection output. Returns [1, n] f32 SBUF
            AP holding rsqrt(mean(x^2) + eps), plus the squares go through a
            bf16 scratch. Also returns a [128, n] f32 PSUM broadcast of it."""
            n = acc_psum.shape[-1]
            sqt = pool_small.tile([128, n], bf, name="sqt", tag="sqt", bufs=2)
            nc.scalar.square(sqt[:], acc_psum)
            zp = ps_z.tile([1, n], f32, name="zp", tag="zp")
            nc.tensor.matmul(zp[:], ones_col[:], sqt[:], start=True, stop=True)
            sq = pool_small.tile([1, n], f32, name="sq", tag="sq", bufs=2)
            nc.scalar.activation(sq[:], zp[:], AF.Sqrt, bias=eps_s[:],
                                 scale=1.0 / DH)
            rn = pool_small.tile([1, n], f32, name="rn", tag="rn", bufs=2)
            nc.vector.reciprocal(rn[:], sq[:])
            nb = ps_b.tile([128, n], f32, name="nb", tag="nb")
            nc.tensor.matmul(nb[:], ones_row[:], rn[:], start=True, stop=True)
            return nb

        # ================= phase T: text k/v =================
        with tc.tile_pool(name="phT", bufs=1) as phT, \
             tc.tile_pool(name="phTt", bufs=2) as phTt:
            ets = phT.tile([128, NDT, L_TXT], bf, name="ets")
            nc.gpsimd.dma_start(
                ets[:], et_d[:, :].rearrange("(t p) l -> p t l", p=128))
            wtvs = phT.tile([128, NDT, HPC * DH], bf, name="wtvs")
            nc.gpsimd.dma_start(
                wtvs[:], wtv_d[:, :].rearrange("(t p) m -> p t m", p=128))
            wtks = phT.tile([128, NDT, HPC * DH], bf, name="wtks")
            nc.gpsimd.dma_start(
                wtks[:], wtk_d[:, :].rearrange("(t p) m -> p t m", p=128))
            for h in range(HPC):
                kp = ps_acc.tile([128, L_TXT], f32, name="kp", tag="acc")
                for d in range(NDT):
                    nc.tensor.matmul(kp[:], wtks[:, d, h * DH:(h + 1) * DH],
                                     ets[:, d, :],
                                     start=(d == 0), stop=(d == NDT - 1))
                ksc = phTt.tile([128, L_TXT], bf, name="ksc", tag="ksc")
                nc.scalar.activation(ksc[:], kp[:], AF.Copy, scale=gtk_s[:])
                nb = rmsnorm_factor(phTt, kp[:])
                nc.vector.tensor_mul(tkf[h][:, :], ksc[:], nb[:])
            for lt in range(NKT_TXT):
                vp = ps_acc.tile([128, HPC * DH], f32, name="vp", tag="acc")
                for d in range(NDT):
                    nc.tensor.matmul(vp[:], ets[:, d, lt * 128:(lt + 1) * 128],
                                     wtvs[:, d, :],
                                     start=(d == 0), stop=(d == NDT - 1))
                nc.scalar.copy(vs[:, lt, :], vp[:])

        # ================= phase P: image q/k/v projections =================
        tc.strict_bb_all_engine_barrier()
        with tc.tile_pool(name="phP", bufs=1) as phP, \
             tc.tile_pool(name="phPx", bufs=2) as phPx, \
             tc.tile_pool(name="phPt", bufs=2) as phPt:
            wqs = phP.tile([128, NDT, HPC * DH], bf, name="wqs")
            nc.gpsimd.dma_start(
                wqs[:], wq_d[:, :].rearrange("(t p) m -> p t m", p=128))
            wks = phP.tile([128, NDT, HPC * DH], bf, name="wks")
            nc.gpsimd.dma_start(
                wks[:], wk_d[:, :].rearrange("(t p) m -> p t m", p=128))
            wvs = phP.tile([128, NDT, HPC * DH], bf, name="wvs")
            nc.gpsimd.dma_start(
                wvs[:], wv_d[:, :].rearrange("(t p) m -> p t m", p=128))
            xt_r = xt_d[:, :].rearrange("(t p) l -> p t l", p=128)
            for lc in range(NLC):
                lsl = slice(lc * 512, (lc + 1) * 512)
                xs = phPx.tile([128, NDT, 512], bf, name="xs", tag="xs")
                nc.gpsimd.dma_start(xs[:], xt_r[:, :, lsl])
                for h in range(HPC):
                    for wt, tab, dst in ((wqs, tqs, qf[h]), (wks, tks_tab, kf[h])):
                        pp = ps_acc.tile([128, 512], f32, name="pp", tag="acc")
                        for d in range(NDT):
                            nc.tensor.matmul(pp[:], wt[:, d, h * DH:(h + 1) * DH],
                                             xs[:, d, :],
                                             start=(d == 0), stop=(d == NDT - 1))
                        ev = phPt.tile([128, 512], bf, name="ev", tag="ev")
                        nc.scalar.copy(ev[:], pp[:])
                        nb = rmsnorm_factor(phPt, pp[:])
                        evn = phPt.tile([128, 512], bf, name="evn", tag="evn")
                        nc.vector.tensor_mul(evn[:], ev[:], nb[:])
                        # rope: dst = tabA*evn + tabB*swap64(evn)
                        evsA = phPt.tile([128, 512], bf, name="evsA", tag="evsA")
                        nc.gpsimd.dma_start(evsA[0:64, :], evn[64:128, :])
                        evsB = phPt.tile([128, 512], bf, name="evsB", tag="evsB")
                        nc.gpsimd.dma_start(evsB[64:128, :], evn[0:64, :])
                        rA = phPt.tile([128, 512], bf, name="rA", tag="rA")
                        nc.vector.tensor_mul(rA[:], evn[:], tab[:, 0, lsl])
                        rB = phPt.tile([128, 512], bf, name="rB", tag="rB")
                        nc.vector.tensor_mul(rB[0:64, :], evsA[0:64, :],
                                             tab[0:64, 1, lsl])
                        nc.vector.tensor_mul(rB[64:128, :], evsB[64:128, :],
                                             tab[64:128, 1, lsl])
                        nc.vector.tensor_add(dst[:, lsl], rA[:], rB[:])
                for ltl in range(4):
                    vp = ps_acc.tile([128, HPC * DH], f32, name="vpi", tag="acc")
                    for d in range(NDT):
                        nc.tensor.matmul(
                            vp[:], xs[:, d, ltl * 128:(ltl + 1) * 128],
                            wvs[:, d, :], start=(d == 0), stop=(d == NDT - 1))
                    nc.scalar.copy(vs[:, NKT_TXT + lc * 4 + ltl, :], vp[:])

        # ============ phase A: attention, phase O: out-projection ============
        tc.strict_bb_all_engine_barrier()
        with tc.tile_pool(name="afbp", bufs=1) as afbp:
            afb = [afbp.tile([DH, L_IMG], bf, name=f"afb{h}", tag=f"afb{h}")
                   for h in range(HPC)]
            with tc.tile_pool(name="phA", bufs=1) as phA, \
                 tc.tile_pool(name="phAt", bufs=2) as phAt:
                pt = phA.tile([128, NKT * 512], bf, name="pt")
                for h in range(HPC):
                    for lqc in range(NLC):
                        qsl = slice(lqc * 512, (lqc + 1) * 512)
                        zp = ps_z.tile([1, 512], f32, name="zpa", tag="zp")
                        av = ps_acc.tile([128, 512], f32, name="av", tag="acc")
                        for lk in range(NKT + PIPE):
                            if lk < NKT:
                                sp = ps_s.tile([128, 512], f32, name="sp", tag="s")
                                if lk < NKT_TXT:
                                    lhsT = tkf[h][:, lk * 128:(lk + 1) * 128]
                                else:
                                    lhsT = kf[h][:, (lk - NKT_TXT) * 128:
                                                 (lk - NKT_TXT + 1) * 128]
                                nc.tensor.matmul(sp[:], lhsT, qf[h][:, qsl],
                                                 start=True, stop=True)
                                nc.scalar.activation(
                                    pt[:, lk * 512:(lk + 1) * 512], sp[:],
                                    AF.Exp, scale=SM_SCALE)
                            j = lk - PIPE
                            if j >= 0:
                                pj = pt[:, j * 512:(j + 1) * 512]
                                nc.tensor.matmul(zp[:], ones_col[:], pj,
                                                 start=(j == 0),
                                                 stop=(j == NKT - 1))
                                nc.tensor.matmul(
                                    av[:], vs[:, j, h * DH:(h + 1) * DH], pj,
                                    start=(j == 0), stop=(j == NKT - 1))
                        zs = phAt.tile([1, 512], f32, name="zs", tag="zs")
                        nc.scalar.add(zs[:], zp[:], nm_s[:])
                        rz = phAt.tile([1, 512], f32, name="rz", tag="rz")
                        nc.vector.reciprocal(rz[:], zs[:])
                        nb2 = ps_b.tile([128, 512], f32, name="nb2", tag="nb")
                        nc.tensor.matmul(nb2[:], ones_row[:], rz[:],
                                         start=True, stop=True)
                        avs = phAt.tile([128, 512], f32, name="avs", tag="avs")
                        nc.scalar.copy(avs[:], av[:])
                        nc.vector.tensor_mul(afb[h][:, qsl], avs[:], nb2[:])

            tc.strict_bb_all_engine_barrier()
            with tc.tile_pool(name="phO", bufs=1) as phO, \
                 tc.tile_pool(name="phOt", bufs=3) as phOt:
                wos = [phO.tile([DH, D], bf, name=f"wos{h}", tag=f"wos{h}")
                       for h in range(HPC)]
                for h in range(HPC):
                    nc.gpsimd.dma_start(wos[h][:], wo_d[h * DH:(h + 1) * DH, :])
                for lt in range(L_IMG // 128):
                    for dc in range(D // 512):
                        op = ps_acc.tile([128, 512], f32, name="op", tag="acc")
                        for hh in range(HPC):
                            nc.tensor.matmul(
                                op[:], afb[hh][:, lt * 128:(lt + 1) * 128],
                                wos[hh][:, dc * 512:(dc + 1) * 512],
                                start=(hh == 0), stop=(hh == HPC - 1))
                        os = phOt.tile([128, 512], f32, name="os", tag="os")
                        nc.scalar.copy(os[:], op[:])
                        nc.gpsimd.dma_start(
                            out_d[lt * 128:(lt + 1) * 128,
                                  dc * 512:(dc + 1) * 512], os[:])

    nc.finalize()
    return nc


def _get_program():
    global _PROG
    if _PROG is None:
        _PROG = _build_program()
    return _PROG


_PERM = np.concatenate([np.arange(0, DH, 2), np.arange(1, DH, 2)])


def make_core_inputs(inputs: dict) -> list:
    hs = np.asarray(inputs["hidden_states"], np.float32)
    enc = np.asarray(inputs["encoder_hidden_states"], np.float32)
    mask = np.asarray(inputs["attention_mask"]).astype(bool)
    emb = np.asarray(inputs["image_rotary_emb"], np.float32)
    wqkv = np.asarray(inputs["w_img_qkv"], np.float32).reshape(D, 3, H, DH)
    wtkv = np.asarray(inputs["w_txt_kv"], np.float32).reshape(D, 2, H, DH)
    wout = np.asarray(inputs["w_out"], np.float32).reshape(H, DH, D)
    g_q = np.asarray(inputs["g_q"], np.float32)
    g_k = np.asarray(inputs["g_k"], np.float32)
    g_ak = np.asarray(inputs["g_added_k"], np.float32)

    def tables(F, g):
        # F: [L, 64, 2, 2]; permuted layout: part p<64 -> dim 2p, 64+p -> 2p+1
        # dst = tabA * evn + tabB * swap64(evn)
        ge, go = g[0::2], g[1::2]
        tabA = np.concatenate([(F[:, :, 0, 0] * ge[None, :]).T,
                               (F[:, :, 1, 1] * go[None, :]).T], axis=0)
        tabB = np.concatenate([(F[:, :, 0, 1] * go[None, :]).T,
                               (F[:, :, 1, 0] * ge[None, :]).T], axis=0)
        return np.stack([tabA, tabB], axis=1).astype(bf16)  # [128, 2, L]

    in_maps = []
    for c in range(NCORES):
        b, g = divmod(c, 4)
        hsel = slice(g * HPC, (g + 1) * HPC)
        F = emb[b, 0]
        wq = wqkv[:, 0, hsel, :][:, :, _PERM].reshape(D, HPC * DH)
        wk = wqkv[:, 1, hsel, :][:, :, _PERM].reshape(D, HPC * DH)
        wv = wqkv[:, 2, hsel, :].reshape(D, HPC * DH)
        wtk = wtkv[:, 0, hsel, :][:, :, _PERM].reshape(D, HPC * DH)
        wtv = wtkv[:, 1, hsel, :].reshape(D, HPC * DH)
        wo = wout[hsel].reshape(HPC * DH, D)
        in_maps.append({
            "xt": np.ascontiguousarray(hs[b].T).astype(bf16),
            "et": np.ascontiguousarray((enc[b] * mask[b][:, None]).T).astype(bf16),
            "wq": np.ascontiguousarray(wq).astype(bf16),
            "wk": np.ascontiguousarray(wk).astype(bf16),
            "wv": np.ascontiguousarray(wv).astype(bf16),
            "wtk": np.ascontiguousarray(wtk).astype(bf16),
            "wtv": np.ascontiguousarray(wtv).astype(bf16),
            "wo": np.ascontiguousarray(wo).astype(bf16),
            "tq": tables(F, g_q),
            "tk": tables(F, g_k),
            "gtk": g_ak[_PERM].reshape(DH, 1).astype(np.float32),
            "nm": np.array([[-(float(L_TXT) - float(mask[b].sum()))]], np.float32),
        })
    return in_maps


def run_cores(in_maps, trace=False, tmpdir=None):
    from concourse.bass_utils import run_bass_kernel_spmd
    nc = _get_program()
    return run_bass_kernel_spmd(nc, in_maps, list(range(NCORES)),
                                trace=trace, tmpdir=tmpdir)


def time_cores(in_maps, iters=30):
    import time

    import jax
    import jax.numpy as jnp
    from jax.sharding import Mesh, PartitionSpec
    from jax.experimental.shard_map import shard_map

    from concourse import bass2jax, mybir

    nc = _get_program()
    bass2jax.install_neuronx_cc_hook()

    partition_name = (nc.partition_id_tensor.name
                      if nc.partition_id_tensor else None)
    in_names, out_names, out_avals, zero_outs = [], [], [], []
    for alloc in nc.m.functions[0].allocations:
        if not isinstance(alloc, mybir.MemoryLocationSet):
            continue
        name = alloc.memorylocations[0].name
        if alloc.kind == "ExternalInput":
            if name != partition_name:
                in_names.append(name)
        elif alloc.kind == "ExternalOutput":
            out_names.append(name)
            shape = tuple(alloc.tensor_shape)
            dtype = mybir.dt.np(alloc.dtype)
            out_avals.append(jax.core.ShapedArray(shape, dtype))
            zero_outs.append(np.zeros(shape, dtype))
    n_params = len(in_names)
    all_names = in_names + out_names
    if partition_name is not None:
        all_names.append(partition_name)

    def _body(*args):
        operands = list(args)
        if partition_name is not None:
            operands.append(bass2jax.partition_id_tensor())
        return tuple(bass2jax._bass_exec_p.bind(
            *operands,
            out_avals=tuple(out_avals),
            in_names=tuple(all_names),
            out_names=tuple(out_names),
            lowering_input_output_aliases=(),
            sim_require_finite=True,
            sim_require_nnan=True,
            nc=nc,
        ))

    devices = jax.devices()[:NCORES]
    mesh = Mesh(np.asarray(devices), ("core",))
    nin = n_params + len(out_names)
    sharded = jax.jit(shard_map(
        _body, mesh=mesh,
        in_specs=(PartitionSpec("core"),) * nin,
        out_specs=(PartitionSpec("core"),) * len(out_names),
        check_rep=False))
    concat_in = [np.concatenate([in_maps[c][nm] for c in range(NCORES)], axis=0)
                 for nm in in_names]
    concat_zero = [np.zeros((NCORES * z.shape[0], *z.shape[1:]), z.dtype)
                   for z in zero_outs]
    sh = jax.sharding.NamedSharding(mesh, PartitionSpec("core"))
    dev_args = [jax.device_put(a, sh) for a in (*concat_in, *concat_zero)]
    out = sharded(*dev_args)
    jax.block_until_ready(out)
    times = []
    for _ in range(iters):
        t0 = time.perf_counter()
        out = sharded(*dev_args)
        jax.block_until_ready(out)
        times.append(time.perf_counter() - t0)
    times_ns = sorted(int(t * 1e9) for t in times)
    return times_ns


def kernel(**inputs) -> np.ndarray:
    in_maps = make_core_inputs(inputs)
    res = run_cores(in_maps)
    out = np.zeros((B, L_IMG, D), np.float32)
    for c in range(NCORES):
        b = c // 4
        out[b] += np.asarray(res.results[c]["out"], np.float32)
    return out


# revision 28
# speedup vs baseline: 194.5064x; 194.5064x over previous
"""PRXAttention TRN2 kernel: 8-core SPMD (2 batches x 4 head-groups).

Per core (b, g): project q/k/v for 4 heads (img) + k/v (txt), RMSNorm via
PE ones-matmul partition reduction, RoPE with host-prepared pair-deinterleaved
tables (g_q/g_k folded in), softmax without max-subtraction (scores bounded),
masking via host-side zeroing of masked encoder tokens + Z correction,
partial out-projection for the 4 heads; host sums the 4 partials per batch.

Softmax Z is accumulated on Pool (even key-tiles) + DVE (odd key-tiles) in
f32, then partition-reduced with two fp32 ones-matmuls; attention and
out-projection are fused per 512-query chunk so PSUM/PE/Act/Pool stay busy.
"""

import numpy as np
import ml_dtypes

bf16 = ml_dtypes.bfloat16

B, L_IMG, L_TXT = 2, 2048, 2048 // 4
D, H, DH = 2048, 16, 128
HPC = 4                      # heads per core
NCORES = 8
EPS = 1e-6
SM_SCALE = 1.0 / float(np.sqrt(DH))
NDT = D // 128               # 16 d-model tiles
NLC = L_IMG // 512           # 4 img l-chunks of 512
NKT_TXT = L_TXT // 128       # 4 txt key tiles
NKT = NKT_TXT + L_IMG // 128  # 20 key tiles of 128
PIPE = 3                     # attention S->exp software pipeline depth

_PROG = {}


def _build_program(reps=1):
    from contextlib import ExitStack

    import concourse.bacc as bacc
    import concourse.bass as bass
    import concourse.tile as tile
    from concourse import mybir

    f32 = mybir.dt.float32
    bf = mybir.dt.bfloat16
    f16 = mybir.dt.float16
    AF = mybir.ActivationFunctionType

    nc = bacc.Bacc("TRN2", target_bir_lowering=False)
    xt_d = nc.declare_dram_parameter("xt", [D, L_IMG], bf, isOutput=False)
    et_d = nc.declare_dram_parameter("et", [D, L_TXT], bf, isOutput=False)
    wq_d = nc.declare_dram_parameter("wq", [D, HPC * DH], bf, isOutput=False)
    wk_d = nc.declare_dram_parameter("wk", [D, HPC * DH], bf, isOutput=False)
    wv_d = nc.declare_dram_parameter("wv", [D, HPC * DH], bf, isOutput=False)
    wtk_d = nc.declare_dram_parameter("wtk", [D, HPC * DH], bf, isOutput=False)
    wtv_d = nc.declare_dram_parameter("wtv", [D, HPC * DH], bf, isOutput=False)
    wo_d = nc.declare_dram_parameter("wo", [HPC * DH, D], bf, isOutput=False)
    tq_d = nc.declare_dram_parameter("tq", [DH, 2, L_IMG], bf, isOutput=False)
    tk_d = nc.declare_dram_parameter("tk", [DH, 2, L_IMG], bf, isOutput=False)
    gtk_d = nc.declare_dram_parameter("gtk", [DH, 1], f32, isOutput=False)
    nm_d = nc.declare_dram_parameter("nm", [1, 1], f32, isOutput=False)
    out_d = nc.declare_dram_parameter("out", [L_IMG, D], f32, isOutput=True)

    with tile.TileContext(nc) as tc, ExitStack() as ctx:
        # ---- persistent pools (whole kernel) ----
        const = ctx.enter_context(tc.tile_pool(name="const", bufs=1))
        persist = ctx.enter_context(tc.tile_pool(name="persist", bufs=1))
        ps_s = ctx.enter_context(
            tc.tile_pool(name="ps_s", bufs=2, space=bass.MemorySpace.PSUM))
        ps_acc = ctx.enter_context(
            tc.tile_pool(name="ps_acc", bufs=2, space=bass.MemorySpace.PSUM))
        ps_z = ctx.enter_context(
            tc.tile_pool(name="ps_z", bufs=1, space=bass.MemorySpace.PSUM))
        ps_b = ctx.enter_context(
            tc.tile_pool(name="ps_b", bufs=1, space=bass.MemorySpace.PSUM))

        ones_col = const.tile([128, 1], bf, name="ones_col")
        nc.vector.memset(ones_col[:], 1.0)
        ones_row_h = const.tile([1, 128], f16, name="ones_row_h")
        nc.vector.memset(ones_row_h[:], 1.0)
        gtk_s = const.tile([DH, 1], f32, name="gtk_s")
        nc.sync.dma_start(gtk_s[:], gtk_d[:, :])
        nm_s = const.tile([1, 1], f32, name="nm_s")
        nc.sync.dma_start(nm_s[:], nm_d[:, :])
        eps_s = const.tile([1, 1], f32, name="eps_s")
        nc.vector.memset(eps_s[:], EPS)
        tqs = const.tile([DH, 2, L_IMG], bf, name="tqs")
        nc.sync.dma_start(tqs[:], tq_d[:, :, :])
        tks_tab = const.tile([DH, 2, L_IMG], bf, name="tks_tab")
        nc.sync.dma_start(tks_tab[:], tk_d[:, :, :])

        qf = [persist.tile([DH, L_IMG], bf, name=f"qf{h}", tag=f"qf{h}")
              for h in range(HPC)]
        kf = [persist.tile([DH, L_IMG], bf, name=f"kf{h}", tag=f"kf{h}")
              for h in range(HPC)]
        tkf = [persist.tile([DH, L_TXT], bf, name=f"tkf{h}", tag=f"tkf{h}")
               for h in range(HPC)]
        vs = persist.tile([128, NKT, HPC * DH], bf, name="vs")

        def rmsnorm_factor(pool_small, acc_psum):
            """acc_psum: [128, n] f32 projection output. Returns a [128, n]
            f32 PSUM broadcast of rsqrt(mean(x^2) + eps) per column."""
            n = acc_psum.shape[-1]
            sqt = pool_small.tile([128, n], bf, name="sqt", tag="sqt", bufs=2)
            nc.scalar.square(sqt[:], acc_psum)
            zp = ps_z.tile([1, n], f32, name="zp", tag="zp")
            nc.tensor.matmul(zp[:], ones_col[:], sqt[:], start=True, stop=True)
            sq = pool_small.tile([1, n], f32, name="sq", tag="sq", bufs=2)
            nc.scalar.activation(sq[:], zp[:], AF.Sqrt, bias=eps_s[:],
                                 scale=1.0 / DH)
            rn = pool_small.tile([1, n], f32, name="rn", tag="rn", bufs=2)
            nc.vector.reciprocal(rn[:], sq[:])
            rnh = pool_small.tile([1, n], f16, name="rnh", tag="rnh", bufs=2)
            nc.scalar.copy(rnh[:], rn[:])
            nb = ps_b.tile([128, n], f32, name="nb", tag="nb")
            nc.tensor.matmul(nb[:], ones_row_h[:], rnh[:], start=True, stop=True)
            return nb

        for _ in range(reps):
            # ================= phase T: text k/v =================
            with tc.tile_pool(name="phT", bufs=1) as phT, \
                 tc.tile_pool(name="phTt", bufs=2) as phTt:
                ets = phT.tile([128, NDT, L_TXT], bf, name="ets")
                nc.sync.dma_start(
                    ets[:], et_d[:, :].rearrange("(t p) l -> p t l", p=128))
                wtvs = phT.tile([128, NDT, HPC * DH], bf, name="wtvs")
                nc.sync.dma_start(
                    wtvs[:], wtv_d[:, :].rearrange("(t p) m -> p t m", p=128))
                wtks = phT.tile([128, NDT, HPC * DH], bf, name="wtks")
                nc.sync.dma_start(
                    wtks[:], wtk_d[:, :].rearrange("(t p) m -> p t m", p=128))
                for h in range(HPC):
                    kp = ps_acc.tile([128, L_TXT], f32, name="kp", tag="acc")
                    for d in range(NDT):
                        nc.tensor.matmul(kp[:], wtks[:, d, h * DH:(h + 1) * DH],
                                         ets[:, d, :],
                                         start=(d == 0), stop=(d == NDT - 1))
                    ksc = phTt.tile([128, L_TXT], bf, name="ksc", tag="ksc")
                    nc.scalar.activation(ksc[:], kp[:], AF.Copy, scale=gtk_s[:])
                    nb = rmsnorm_factor(phTt, kp[:])
                    nc.vector.tensor_mul(tkf[h][:, :], ksc[:], nb[:])
                for lt in range(NKT_TXT):
                    vp = ps_acc.tile([128, HPC * DH], f32, name="vp", tag="acc")
                    for d in range(NDT):
                        nc.tensor.matmul(vp[:],
                                         ets[:, d, lt * 128:(lt + 1) * 128],
                                         wtvs[:, d, :],
                                         start=(d == 0), stop=(d == NDT - 1))
                    nc.scalar.copy(vs[:, lt, :], vp[:])

            # ============== phase P: image q/k/v projections ==============
            with tc.tile_pool(name="phP", bufs=1) as phP, \
                 tc.tile_pool(name="phPx", bufs=2) as phPx, \
                 tc.tile_pool(name="phPt", bufs=2) as phPt:
                wqs = phP.tile([128, NDT, HPC * DH], bf, name="wqs")
                nc.sync.dma_start(
                    wqs[:], wq_d[:, :].rearrange("(t p) m -> p t m", p=128))
                wks = phP.tile([128, NDT, HPC * DH], bf, name="wks")
                nc.sync.dma_start(
                    wks[:], wk_d[:, :].rearrange("(t p) m -> p t m", p=128))
                wvs = phP.tile([128, NDT, HPC * DH], bf, name="wvs")
                nc.sync.dma_start(
                    wvs[:], wv_d[:, :].rearrange("(t p) m -> p t m", p=128))
                xt_r = xt_d[:, :].rearrange("(t p) l -> p t l", p=128)
                for lc in range(NLC):
                    lsl = slice(lc * 512, (lc + 1) * 512)
                    xs = phPx.tile([128, NDT, 512], bf, name="xs", tag="xs")
                    nc.sync.dma_start(xs[:], xt_r[:, :, lsl])
                    for h in range(HPC):
                        for wt, tab, dst in ((wqs, tqs, qf[h]),
                                             (wks, tks_tab, kf[h])):
                            pp = ps_acc.tile([128, 512], f32, name="pp",
                                             tag="acc")
                            for d in range(NDT):
                                nc.tensor.matmul(
                                    pp[:], wt[:, d, h * DH:(h + 1) * DH],
                                    xs[:, d, :],
                                    start=(d == 0), stop=(d == NDT - 1))
                            ev = phPt.tile([128, 512], bf, name="ev", tag="ev")
                            nc.scalar.copy(ev[:], pp[:])
                            nb = rmsnorm_factor(phPt, pp[:])
                            # rope then norm: dst = (tabA*ev + tabB*swap64(ev))*nb
                            evsA = phPt.tile([128, 512], bf, name="evsA",
                                             tag="evsA")
                            nc.sync.dma_start(evsA[0:64, :], ev[64:128, :])
                            evsB = phPt.tile([128, 512], bf, name="evsB",
                                             tag="evsB")
                            nc.sync.dma_start(evsB[64:128, :], ev[0:64, :])
                            rA = phPt.tile([128, 512], bf, name="rA", tag="rA")
                            nc.gpsimd.tensor_mul(rA[:], ev[:], tab[:, 0, lsl])
                            rB = phPt.tile([128, 512], bf, name="rB", tag="rB")
                            nc.vector.tensor_mul(rB[0:64, :], evsA[0:64, :],
                                                 tab[0:64, 1, lsl])
                            nc.vector.tensor_mul(rB[64:128, :], evsB[64:128, :],
                                                 tab[64:128, 1, lsl])
                            rs = phPt.tile([128, 512], bf, name="rs", tag="rs")
                            nc.gpsimd.tensor_add(rs[:], rA[:], rB[:])
                            nc.vector.tensor_mul(dst[:, lsl], rs[:], nb[:])
                    for ltl in range(4):
                        vp = ps_acc.tile([128, HPC * DH], f32, name="vpi",
                                         tag="acc")
                        for d in range(NDT):
                            nc.tensor.matmul(
                                vp[:], xs[:, d, ltl * 128:(ltl + 1) * 128],
                                wvs[:, d, :], start=(d == 0),
                                stop=(d == NDT - 1))
                        nc.scalar.copy(vs[:, NKT_TXT + lc * 4 + ltl, :], vp[:])

            # ====== phase A+O: attention fused with out-projection ======
            with tc.tile_pool(name="phA", bufs=1) as phA, \
                 tc.tile_pool(name="phAt", bufs=2) as phAt, \
                 tc.tile_pool(name="phAv", bufs=2) as phAv, \
                 tc.tile_pool(name="phOt", bufs=3) as phOt:
                wos = [phA.tile([DH, D], bf, name=f"wos{h}", tag=f"wos{h}")
                       for h in range(HPC)]
                for h in range(HPC):
                    nc.sync.dma_start(wos[h][:], wo_d[h * DH:(h + 1) * DH, :])
                pt = phA.tile([128, NKT * 512], bf, name="pt")
                for lqc in range(NLC):
                    qsl = slice(lqc * 512, (lqc + 1) * 512)
                    afv = phAv.tile([128, HPC, 512], bf, name="afv", tag="afv")
                    for h in range(HPC):
                        paP = phAt.tile([128, 512], f32, name="paP", tag="paP")
                        paD = phAt.tile([128, 512], f32, name="paD", tag="paD")
                        av = ps_acc.tile([128, 512], f32, name="av", tag="acc")
                        for lkp in range(NKT // 2 + 1):
                            if lkp < NKT // 2:
                                spw = ps_s.tile([128, 1024], f32, name="spw",
                                                tag="s")
                                for half in range(2):
                                    lk = 2 * lkp + half
                                    if lk < NKT_TXT:
                                        lhsT = tkf[h][:, lk * 128:
                                                      (lk + 1) * 128]
                                    else:
                                        lhsT = kf[h][:, (lk - NKT_TXT) * 128:
                                                     (lk - NKT_TXT + 1) * 128]
                                    nc.tensor.matmul(
                                        spw[:, half * 512:(half + 1) * 512],
                                        lhsT, qf[h][:, qsl],
                                        start=True, stop=True)
                                nc.scalar.activation(
                                    pt[:, lkp * 1024:(lkp + 1) * 1024],
                                    spw[:], AF.Exp, scale=SM_SCALE)
                            jp = lkp - 1
                            if jp >= 0:
                                for half in range(2):
                                    j = 2 * jp + half
                                    pj = pt[:, j * 512:(j + 1) * 512]
                                    nc.tensor.matmul(
                                        av[:], vs[:, j, h * DH:(h + 1) * DH],
                                        pj, start=(j == 0),
                                        stop=(j == NKT - 1))
                                    if j % 2 == 0:
                                        if j == 0:
                                            nc.gpsimd.tensor_copy(paP[:], pj)
                                        else:
                                            nc.gpsimd.tensor_add(paP[:],
                                                                 paP[:], pj)
                                    else:
                                        if j == 1:
                                            nc.vector.tensor_copy(paD[:], pj)
                                        else:
                                            nc.vector.tensor_add(paD[:],
                                                                 paD[:], pj)
                        paS = phAt.tile([128, 512], bf, name="paS", tag="paS")
                        nc.vector.tensor_add(paS[:], paP[:], paD[:])
                        zp = ps_z.tile([1, 512], f32, name="zpa", tag="zp")
                        nc.tensor.matmul(zp[:], ones_col[:], paS[:],
                                         start=True, stop=True)
                        zs = phAt.tile([1, 512], f32, name="zs", tag="zs")
                        nc.scalar.add(zs[:], zp[:], nm_s[:])
                        rz = phAt.tile([1, 512], f32, name="rz", tag="rz")
                        nc.vector.reciprocal(rz[:], zs[:])
                        rzh = phAt.tile([1, 512], f16, name="rzh", tag="rzh")
                        nc.scalar.copy(rzh[:], rz[:])
                        nb2 = ps_b.tile([128, 512], f32, name="nb2", tag="nb")
                        nc.tensor.matmul(nb2[:], ones_row_h[:], rzh[:],
                                         start=True, stop=True)
                        avs = phAt.tile([128, 512], f32, name="avs", tag="avs")
                        nc.scalar.copy(avs[:], av[:])
                        nc.vector.tensor_mul(afv[:, h, :], avs[:], nb2[:])
                    for ltl in range(4):
                        row0 = lqc * 512 + ltl * 128
                        for dc in range(D // 512):
                            op = ps_acc.tile([128, 512], f32, name="op",
                                             tag="acc")
                            for hh in range(HPC):
                                nc.tensor.matmul(
                                    op[:],
                                    afv[:, hh, ltl * 128:(ltl + 1) * 128],
                                    wos[hh][:, dc * 512:(dc + 1) * 512],
                                    start=(hh == 0), stop=(hh == HPC - 1))
                            os_t = phOt.tile([128, 512], f32, name="os",
                                             tag="os")
                            nc.vector.tensor_copy(os_t[:], op[:])
                            nc.sync.dma_start(
                                out_d[row0:row0 + 128,
                                      dc * 512:(dc + 1) * 512], os_t[:])

    nc.finalize()
    return nc


def _get_program(reps=1):
    if reps not in _PROG:
        _PROG[reps] = _build_program(reps=reps)
    return _PROG[reps]


_PERM = np.concatenate([np.arange(0, DH, 2), np.arange(1, DH, 2)])


def make_core_inputs(inputs: dict) -> list:
    hs = np.asarray(inputs["hidden_states"], np.float32)
    enc = np.asarray(inputs["encoder_hidden_states"], np.float32)
    mask = np.asarray(inputs["attention_mask"]).astype(bool)
    emb = np.asarray(inputs["image_rotary_emb"], np.float32)
    wqkv = np.asarray(inputs["w_img_qkv"], np.float32).reshape(D, 3, H, DH)
    wtkv = np.asarray(inputs["w_txt_kv"], np.float32).reshape(D, 2, H, DH)
    wout = np.asarray(inputs["w_out"], np.float32).reshape(H, DH, D)
    g_q = np.asarray(inputs["g_q"], np.float32)
    g_k = np.asarray(inputs["g_k"], np.float32)
    g_ak = np.asarray(inputs["g_added_k"], np.float32)

    def tables(F, g):
        # F: [L, 64, 2, 2]; permuted layout: part p<64 -> dim 2p, 64+p -> 2p+1
        # dst = tabA * ev + tabB * swap64(ev)
        ge, go = g[0::2], g[1::2]
        tabA = np.concatenate([(F[:, :, 0, 0] * ge[None, :]).T,
                               (F[:, :, 1, 1] * go[None, :]).T], axis=0)
        tabB = np.concatenate([(F[:, :, 0, 1] * go[None, :]).T,
                               (F[:, :, 1, 0] * ge[None, :]).T], axis=0)
        return np.stack([tabA, tabB], axis=1).astype(bf16)  # [128, 2, L]

    in_maps = []
    for c in range(NCORES):
        b, g = divmod(c, 4)
        hsel = slice(g * HPC, (g + 1) * HPC)
        F = emb[b, 0]
        wq = wqkv[:, 0, hsel, :][:, :, _PERM].reshape(D, HPC * DH)
        wk = wqkv[:, 1, hsel, :][:, :, _PERM].reshape(D, HPC * DH)
        wv = wqkv[:, 2, hsel, :].reshape(D, HPC * DH)
        wtk = wtkv[:, 0, hsel, :][:, :, _PERM].reshape(D, HPC * DH)
        wtv = wtkv[:, 1, hsel, :].reshape(D, HPC * DH)
        wo = wout[hsel].reshape(HPC * DH, D)
        in_maps.append({
            "xt": np.ascontiguousarray(hs[b].T).astype(bf16),
            "et": np.ascontiguousarray((enc[b] * mask[b][:, None]).T).astype(bf16),
            "wq": np.ascontiguousarray(wq).astype(bf16),
            "wk": np.ascontiguousarray(wk).astype(bf16),
            "wv": np.ascontiguousarray(wv).astype(bf16),
            "wtk": np.ascontiguousarray(wtk).astype(bf16),
            "wtv": np.ascontiguousarray(wtv).astype(bf16),
            "wo": np.ascontiguousarray(wo).astype(bf16),
            "tq": tables(F, g_q),
            "tk": tables(F, g_k),
            "gtk": g_ak[_PERM].reshape(DH, 1).astype(np.float32),
            "nm": np.array([[-(float(L_TXT) - float(mask[b].sum()))]], np.float32),
        })
    return in_maps


def run_cores(in_maps, trace=False, tmpdir=None):
    from concourse.bass_utils import run_bass_kernel_spmd
    nc = _get_program()
    return run_bass_kernel_spmd(nc, in_maps, list(range(NCORES)),
                                trace=trace, tmpdir=tmpdir)


def time_cores(in_maps, iters=30, reps=1, ncores=NCORES):
    import time

    import jax
    from jax.sharding import Mesh, PartitionSpec
    from jax.experimental.shard_map import shard_map

    from concourse import bass2jax, mybir

    nc = _get_program(reps)
    bass2jax.install_neuronx_cc_hook()

    partition_name = (nc.partition_id_tensor.name
                      if nc.partition_id_tensor else None)
    in_names, out_names, out_avals, zero_outs = [], [], [], []
    for alloc in nc.m.functions[0].allocations:
        if not isinstance(alloc, mybir.MemoryLocationSet):
            continue
        name = alloc.memorylocations[0].name
        if alloc.kind == "ExternalInput":
            if name != partition_name:
                in_names.append(name)
        elif alloc.kind == "ExternalOutput":
            out_names.append(name)
            shape = tuple(alloc.tensor_shape)
            dtype = mybir.dt.np(alloc.dtype)
            out_avals.append(jax.core.ShapedArray(shape, dtype))
            zero_outs.append(np.zeros(shape, dtype))
    n_params = len(in_names)
    all_names = in_names + out_names
    if partition_name is not None:
        all_names.append(partition_name)

    def _body(*args):
        operands = list(args)
        if partition_name is not None:
            operands.append(bass2jax.partition_id_tensor())
        return tuple(bass2jax._bass_exec_p.bind(
            *operands,
            out_avals=tuple(out_avals),
            in_names=tuple(all_names),
            out_names=tuple(out_names),
            lowering_input_output_aliases=(),
            sim_require_finite=True,
            sim_require_nnan=True,
            nc=nc,
        ))

    devices = jax.devices()[:ncores]
    mesh = Mesh(np.asarray(devices), ("core",))
    nin = n_params + len(out_names)
    sharded = jax.jit(shard_map(
        _body, mesh=mesh,
        in_specs=(PartitionSpec("core"),) * nin,
        out_specs=(PartitionSpec("core"),) * len(out_names),
        check_rep=False))
    concat_in = [np.concatenate([in_maps[c][nm] for c in range(ncores)], axis=0)
                 for nm in in_names]
    concat_zero = [np.zeros((ncores * z.shape[0], *z.shape[1:]), z.dtype)
                   for z in zero_outs]
    sh = jax.sharding.NamedSharding(mesh, PartitionSpec("core"))
    dev_args = [jax.device_put(a, sh) for a in (*concat_in, *concat_zero)]
    out = sharded(*dev_args)
    jax.block_until_ready(out)
    times = []
    for _ in range(iters):
        t0 = time.perf_counter()
        out = sharded(*dev_args)
        jax.block_until_ready(out)
        times.append(time.perf_counter() - t0)
    times_ns = sorted(int(t * 1e9) for t in times)
    return times_ns


def kernel(**inputs) -> np.ndarray:
    in_maps = make_core_inputs(inputs)
    res = run_cores(in_maps)
    out = np.zeros((B, L_IMG, D), np.float32)
    for c in range(NCORES):
        b = c // 4
        out[b] += np.asarray(res.results[c]["out"], np.float32)
    return out
